# revision 2
# baseline (speedup 1.0000x reference)
"""Trainium2 Bass kernel v2 for nn_GRNNTransformGated.

Changes vs baseline:
  - z computed as softmax-logit DIFFS vs the u-gate pivot: 3 gate blocks
    instead of 4 (saves 4 matmul blocks + 1 exp per tile); combine
    restructured to out = (u + f0*hH + fL*hL + fR*hR) / (1+f0+fL+fR).
  - fp8 e4m3 DoubleRow matmuls for the r-gates and z-diff gates (2x PE
    throughput): pairs (hL,hR) ride the naturally interleaved child
    layout; (hH,u) pair lives in a dedicated 2-lane fp8 tile. Weights
    scaled x16 into fp8 range; unscaled for free via ACT scale.
  - emb levels stored twice: bf16 (value path) + fp8 (matmul path).
  - All elementwise intermediates bf16 in SBUF for DVE 2x modes; no
    scalar_tensor_tensor on hot paths (runs 1x on HW) — the (t+1) fold
    rides 3 extra h-matmul accumulate blocks instead.
  - u-conv emitted as a separate per-level pass (no child dependency)
    to keep the PE fed; conv bias folded into a ones-row of ct.
  - Engine map: ACT = tanh/exp (L/R-paired over 2-bank PSUM) + h-relu +
    fp8 out-casts; DVE = everything else; GPSIMD idle (its ~1.5us/op
    latency regressed the combine tail wherever tried).
"""

import sys

for _p in ("/opt/trn_rl_repo", "/root/.axon_site/_ro/trn_rl_repo"):
    if _p not in sys.path:
        sys.path.insert(0, _p)

import numpy as np

B = 1024
L = 10
H = 128
FEAT = 7
NCORES = 8
TPC = B // NCORES          # trees per core = 128
TCH = 16                   # trees per chunk
NCHUNK = TPC // TCH        # 8 chunks
NPC = TPC * (2 ** L - 1)   # nodes per core = 130944
LOFF = [TPC * (2 ** j - 1) for j in range(L + 1)]
LEVEL_SIZES = [B * 2 ** j for j in range(L)]
OFF = np.concatenate([[0], np.cumsum(LEVEL_SIZES)]).astype(int)
INNER = LEVEL_SIZES[:-1]
COFF = np.concatenate([[0], np.cumsum(INNER)]).astype(int)

MMT = 512
SW = 16.0   # fp8 weight scale
USE_FP8 = True

_CACHE = {}


def _children_canonical(children):
    for j in range(L - 1):
        n = INNER[j]
        blk = children[COFF[j]:COFF[j + 1]]
        base = 2 * np.arange(n, dtype=np.int64)
        if not (np.array_equal(blk[:, 0], base) and np.array_equal(blk[:, 1], base + 1)):
            return False
    return True


def _numpy_fallback(contents, children, W_u, b_u, W_h, b_h, W_z, b_z, W_r, b_r,
                    conv_w, conv_b):
    w, b = float(conv_w[0]), float(conv_b[0])

    def conv_chain(x):
        for _ in range(3):
            x = np.maximum(w * x + b, 0.0)
        return x

    def sigmoid(x):
        return 1.0 / (1.0 + np.exp(-x))

    emb = None
    for j in reversed(range(L)):
        c = contents[OFF[j]:OFF[j + 1]]
        u = conv_chain(c @ W_u + b_u)
        if j == L - 1:
            emb = u
            continue
        ch = children[COFF[j]:COFF[j + 1]]
        h_L = emb[ch[:, 0]]
        h_R = emb[ch[:, 1]]
        hhu = np.concatenate([h_L, h_R, u], axis=1)
        r = sigmoid(hhu @ W_r + b_r)
        h_H = conv_chain((r * hhu) @ W_h + b_h)
        z = np.concatenate([h_H, hhu], axis=1) @ W_z + b_z
        zs = np.stack([z[:, :H], z[:, H:2 * H], z[:, 2 * H:3 * H], z[:, 3 * H:]], axis=-1)
        zs = zs - zs.max(axis=-1, keepdims=True)
        e = np.exp(zs)
        g = e / e.sum(axis=-1, keepdims=True)
        emb = g[..., 0] * h_H + g[..., 1] * h_L + g[..., 2] * h_R + g[..., 3] * u
    return emb.reshape(B, -1).astype(np.float32)


def _build(cw, cb, tanh_pair, exp_pair):
    from contextlib import ExitStack

    from concourse import bacc, mybir, tile

    f32 = mybir.dt.float32
    bf16 = mybir.dt.bfloat16
    f8 = mybir.dt.float8e4 if USE_FP8 else mybir.dt.bfloat16
    AF = mybir.ActivationFunctionType
    OP = mybir.AluOpType
    DR = mybir.MatmulPerfMode.DoubleRow
    AX = mybir.AxisListType

    A = cw * cw
    C = cw * cb + cb

    nc = bacc.Bacc()

    ct_d = nc.declare_dram_parameter("ct", [FEAT + 1, NPC], bf16, isOutput=False)
    wu_d = nc.declare_dram_parameter("wu", [FEAT + 1, H], bf16, isOutput=False)
    wrlr_d = nc.declare_dram_parameter("wrlr", [H, 3, 2, H], f8, isOutput=False)
    wru_d = nc.declare_dram_parameter("wru", [H, 3, H], f8, isOutput=False)
    wdlr_d = nc.declare_dram_parameter("wdlr", [H, 3, 2, H], f8, isOutput=False)
    wdhu_d = nc.declare_dram_parameter("wdhu", [H, 3, 2, H], f8, isOutput=False)
    wh_d = nc.declare_dram_parameter("wh", [H, 3, H], bf16, isOutput=False)
    bv_d = nc.declare_dram_parameter("bvec", [H, 8], f32, isOutput=False)
    id_d = nc.declare_dram_parameter("ident", [H, H], f32, isOutput=False)
    out_d = nc.declare_dram_parameter("out", [TPC, H], f32, isOutput=True)

    with ExitStack() as ctx:
        tc = ctx.enter_context(tile.TileContext(nc))
        wpool = ctx.enter_context(tc.tile_pool(name="wts", bufs=1))
        epool = ctx.enter_context(tc.tile_pool(name="emb", bufs=1))
        ctpool = ctx.enter_context(tc.tile_pool(name="ct", bufs=5))
        upool = ctx.enter_context(tc.tile_pool(name="ub", bufs=10))
        u8pool = ctx.enter_context(tc.tile_pool(name="u8", bufs=10))
        tpool = ctx.enter_context(tc.tile_pool(name="tt", bufs=3))
        rpool = ctx.enter_context(tc.tile_pool(name="rh", bufs=3))
        fpool = ctx.enter_context(tc.tile_pool(name="fl", bufs=3))
        spool = ctx.enter_context(tc.tile_pool(name="tmp", bufs=3))
        ppA = ctx.enter_context(tc.tile_pool(name="ppA", bufs=4, space="PSUM"))
        ppB2 = ctx.enter_context(tc.tile_pool(name="ppB2", bufs=2, space="PSUM"))
        ppB1 = ppA

        wu = wpool.tile([FEAT + 1, H], bf16, name="wu", tag="wu")
        wrlr = wpool.tile([H, 3, 2, H], f8, name="wrlr", tag="wrlr")
        wru = wpool.tile([H, 3, H], f8, name="wru", tag="wru")
        wdlr = wpool.tile([H, 3, 2, H], f8, name="wdlr", tag="wdlr")
        wdhu = wpool.tile([H, 3, 2, H], f8, name="wdhu", tag="wdhu")
        wh = wpool.tile([H, 3, H], bf16, name="wh", tag="wh")
        bv = wpool.tile([H, 8], f32, name="bv", tag="bv")
        idt = wpool.tile([H, H], f32, name="idt", tag="idt")
        for t, d in ((wu, wu_d), (wrlr, wrlr_d), (wru, wru_d), (wdlr, wdlr_d),
                     (wdhu, wdhu_d), (wh, wh_d), (bv, bv_d), (idt, id_d)):
            nc.sync.dma_start(t[:], d[:])

        # emb level buffers, bf16 + fp8 mirrors
        def elvl(name, n, mirror=True):
            eb = epool.tile([H, n], bf16, name=name, tag=name)
            e8 = (epool.tile([H, n], f8, name=name + "_8", tag=name + "_8")
                  if mirror else None)
            return eb, e8

        e9, e9m = elvl("e9", TCH * 512)
        e8, e8m = elvl("e8", TCH * 256)
        e7, e7m = elvl("e7", TCH * 128)
        e6, e6m = elvl("e6", TCH * 64)
        em5, em5m = elvl("em5", TPC * 32)
        e4, e4m = elvl("e4", TPC * 16)
        e3, e3m = elvl("e3", TPC * 8)
        e2, e2m = elvl("e2", TPC * 4)
        e1, e1m = elvl("e1", TPC * 2)
        e0f = epool.tile([H, TPC], f32, name="e0f", tag="e0f")

        def leaf_tile(ct_ap, out_bf, out_f8, n):
            pu = ppA.tile([H, MMT], f32, name="pu", tag="pa")
            nc.tensor.matmul(pu[:, :n], wu[:], ct_ap, start=True, stop=True)
            nc.vector.tensor_scalar(out_bf, pu[:, :n], 0.0, C, OP.max, OP.add)
            nc.scalar.copy(out_f8, out_bf)

        def u_pass(ct_ap, n):
            # u-conv pipeline: depends only on ct, so it can run a whole
            # level ahead and keep the PE fed during elementwise phases
            pu = ppA.tile([H, MMT], f32, name="pu", tag="pa")
            nc.tensor.matmul(pu[:, :n], wu[:], ct_ap, start=True, stop=True)
            ub = upool.tile([H, MMT], bf16, name="ub", tag="ub")
            nc.vector.tensor_scalar(ub[:, :n], pu[:, :n], 0.0, C, OP.max, OP.add)
            u8v = u8pool.tile([H, 2, MMT], f8, name="u8v", tag="u8v")   # lane0=hH8, lane1=u8
            nc.vector.tensor_copy(u8v[:, 1, :n], ub[:, :n])
            return ub, u8v

        def stage_a(cbuf, cbuf8, pbase, n, ub, u8v):
            c2 = 2 * pbase
            csl = cbuf[:, c2:c2 + 2 * n]                    # [H, 2n] bf16
            cpair8 = cbuf8[:, c2:c2 + 2 * n].rearrange("p (n two) -> p two n", two=2)
            # ---- r gate logits ----
            prLR = ppB2.tile([H, 2, MMT], f32, name="prLR", tag="pb2")
            pru = ppB1.tile([H, MMT], f32, name="pru", tag="pa")
            for m, pdst in ((0, prLR[:, 0, :n]), (1, prLR[:, 1, :n]), (2, pru[:, :n])):
                if USE_FP8:
                    nc.tensor.matmul(pdst, wrlr[:, m], cpair8[:, :, :n], start=True,
                                     stop=False, perf_mode=DR, skip_group_check=True)
                else:
                    nc.tensor.matmul(pdst, wrlr[:, m, 0], cpair8[:, 0, :n],
                                     start=True, stop=False, skip_group_check=True)
                    nc.tensor.matmul(pdst, wrlr[:, m, 1], cpair8[:, 1, :n],
                                     start=False, stop=False, skip_group_check=True)
                nc.tensor.matmul(pdst, wru[:, m], u8v[:, 1, :n], start=False,
                                 stop=True, skip_group_check=True)
            # ---- tanh (r as 2*sigma-1) ----
            tt = tpool.tile([H, MMT, 2], bf16, name="tt", tag="tt")
            tu = tpool.tile([H, MMT], bf16, name="tu", tag="tu")
            if tanh_pair:
                nc.scalar.activation(tt[:, :n, :].rearrange("p n c -> p c n"),
                                     prLR[:, :, :n], AF.Tanh,
                                     bias=bv[:, 0:1], scale=0.5 / SW)
            else:
                nc.scalar.activation(tt[:, :n, 0], prLR[:, 0, :n], AF.Tanh,
                                     bias=bv[:, 0:1], scale=0.5 / SW)
                nc.scalar.activation(tt[:, :n, 1], prLR[:, 1, :n], AF.Tanh,
                                     bias=bv[:, 1:2], scale=0.5 / SW)
            nc.scalar.activation(tu[:, :n], pru[:, :n], AF.Tanh,
                                 bias=bv[:, 2:3], scale=0.5 / SW)
            # ---- rh = (t+1)*hhu  (0.5 folded into wh) ----
            rhh = rpool.tile([H, MMT, 2], bf16, name="rhh", tag="rhh")
            rhu = rpool.tile([H, MMT], bf16, name="rhu", tag="rhu")
            rhh2 = rhh[:].rearrange("p n c -> p (n c)")
            tt2 = tt[:].rearrange("p n c -> p (n c)")
            nc.vector.tensor_tensor(rhh2[:, :2 * n], tt2[:, :2 * n], csl, OP.mult)
            nc.vector.tensor_tensor(rhu[:, :n], tu[:, :n], ub[:, :n], OP.mult)
            return rhh, rhu

        def stage_b(cbuf, cbuf8, pbase, n, ub, u8v, rhh, rhu, out_ap, out8_ap,
                    next_a):
            c2 = 2 * pbase
            csl = cbuf[:, c2:c2 + 2 * n]                    # [H, 2n] bf16
            csl3 = csl.rearrange("p (n two) -> p n two", two=2)
            cpair8 = cbuf8[:, c2:c2 + 2 * n].rearrange("p (n two) -> p two n", two=2)
            # ---- h ----
            rh3 = rhh[:].rearrange("p n two -> p two n")
            ph = ppA.tile([H, MMT], f32, name="ph", tag="pa")
            nc.tensor.matmul(ph[:, :n], wh[:, 0], rh3[:, 0, :n], start=True, stop=False)
            nc.tensor.matmul(ph[:, :n], wh[:, 1], rh3[:, 1, :n], start=False, stop=False)
            nc.tensor.matmul(ph[:, :n], wh[:, 2], rhu[:, :n], start=False, stop=False)
            nc.tensor.matmul(ph[:, :n], wh[:, 0], csl3[:, :n, 0], start=False, stop=False)
            nc.tensor.matmul(ph[:, :n], wh[:, 1], csl3[:, :n, 1], start=False, stop=False)
            nc.tensor.matmul(ph[:, :n], wh[:, 2], ub[:, :n], start=False, stop=True)
            hm = upool.tile([H, MMT], bf16, name="hm", tag="hm")       # hH' (sans C)
            nc.scalar.activation(hm[:, :n], ph[:, :n], AF.Relu, bias=bv[:, 6:7])
            hc = upool.tile([H, MMT], bf16, name="hc", tag="hc")       # hH (with C)
            nc.vector.tensor_scalar_add(hc[:, :n], hm[:, :n], C)
            nc.vector.tensor_copy(u8v[:, 0, :n], hc[:, :n])
            if next_a is not None:
                next_a()   # tile t+1's r/tanh/rh fill the hH8-wait bubble
            # ---- z diff logits (pivot = u gate) ----
            pzLR = ppB2.tile([H, 2, MMT], f32, name="pzLR", tag="pb2")
            pz0 = ppB1.tile([H, MMT], f32, name="pz0", tag="pa")
            for m, pdst in ((0, pz0[:, :n]), (1, pzLR[:, 0, :n]), (2, pzLR[:, 1, :n])):
                if USE_FP8:
                    nc.tensor.matmul(pdst, wdlr[:, m], cpair8[:, :, :n], start=True,
                                     stop=False, perf_mode=DR, skip_group_check=True)
                    nc.tensor.matmul(pdst, wdhu[:, m], u8v[:, :, :n], start=False,
                                     stop=True, perf_mode=DR, skip_group_check=True)
                else:
                    nc.tensor.matmul(pdst, wdlr[:, m, 0], cpair8[:, 0, :n],
                                     start=True, stop=False, skip_group_check=True)
                    nc.tensor.matmul(pdst, wdlr[:, m, 1], cpair8[:, 1, :n],
                                     start=False, stop=False, skip_group_check=True)
                    nc.tensor.matmul(pdst, wdhu[:, m, 0], u8v[:, 0, :n],
                                     start=False, stop=False, skip_group_check=True)
                    nc.tensor.matmul(pdst, wdhu[:, m, 1], u8v[:, 1, :n],
                                     start=False, stop=True, skip_group_check=True)
            # ---- exp ----
            fl = fpool.tile([H, MMT, 2], bf16, name="fl", tag="fl")
            f0 = fpool.tile([H, MMT], bf16, name="f0", tag="f0")
            nc.scalar.activation(f0[:, :n], pz0[:, :n], AF.Exp,
                                 bias=bv[:, 3:4], scale=1.0 / SW)
            if exp_pair:
                nc.scalar.activation(fl[:, :n, :].rearrange("p n c -> p c n"),
                                     pzLR[:, :, :n], AF.Exp,
                                     bias=bv[:, 4:5], scale=1.0 / SW)
            else:
                nc.scalar.activation(fl[:, :n, 0], pzLR[:, 0, :n], AF.Exp,
                                     bias=bv[:, 4:5], scale=1.0 / SW)
                nc.scalar.activation(fl[:, :n, 1], pzLR[:, 1, :n], AF.Exp,
                                     bias=bv[:, 5:6], scale=1.0 / SW)
            # ---- combine ----
            mlr = spool.tile([H, MMT, 2], bf16, name="mlr", tag="mlr")
            mlr2 = mlr[:].rearrange("p n c -> p (n c)")
            fl2 = fl[:].rearrange("p n c -> p (n c)")
            nc.vector.tensor_tensor(mlr2[:, :2 * n], fl2[:, :2 * n], csl, OP.mult)
            mS = spool.tile([H, MMT], bf16, name="mS", tag="mS")
            nc.gpsimd.tensor_tensor(mS[:, :n], mlr[:, :n, 0], mlr[:, :n, 1], OP.add)
            m0 = spool.tile([H, MMT], bf16, name="m0", tag="m0")
            nc.vector.tensor_tensor(m0[:, :n], hc[:, :n], f0[:, :n], OP.mult)
            a1 = spool.tile([H, MMT], bf16, name="a1", tag="a1")
            nc.vector.tensor_tensor(a1[:, :n], ub[:, :n], m0[:, :n], OP.add)
            a2 = spool.tile([H, MMT], bf16, name="a2", tag="a2")
            nc.vector.tensor_tensor(a2[:, :n], a1[:, :n], mS[:, :n], OP.add)
            fS = spool.tile([H, MMT], bf16, name="fS", tag="fS")
            nc.gpsimd.tensor_tensor(fS[:, :n], fl[:, :n, 0], fl[:, :n, 1], OP.add)
            den = spool.tile([H, MMT], f32, name="den", tag="den")
            nc.vector.scalar_tensor_tensor(den[:, :n], f0[:, :n], 1.0, fS[:, :n],
                                           OP.add, OP.add)
            rcp = spool.tile([H, MMT], f32, name="rcp", tag="rcp")
            nc.vector.reciprocal_approx_fast(rcp[:, :n], den[:, :n])
            nc.vector.tensor_tensor(out_ap, a2[:, :n], rcp[:, :n], OP.mult)
            if out8_ap is not None:
                nc.scalar.copy(out8_ap, out_ap)

        def run_level(nj, ct_base, cb, cb8, ob, ob8):
            # pass 1: u-convs for the whole level (no child dependency)
            tiles = []
            done = 0
            while done < nj:
                piece = min(2048, nj - done)
                ctt = ctpool.tile([FEAT + 1, 2048], bf16, name="ctt", tag="ctt")
                nc.sync.dma_start(ctt[:, :piece],
                                  ct_d[:, ct_base + done:ct_base + done + piece])
                for s in range(0, piece, MMT):
                    n = min(MMT, piece - s)
                    ub, u8v = u_pass(ctt[:, s:s + n], n)
                    tiles.append((done + s, n, ub, u8v))
                done += piece
            # pass 2: software-pipelined, stage A runs one tile ahead
            rhs = [None] * len(tiles)
            rhs[0] = stage_a(cb, cb8, tiles[0][0], tiles[0][1],
                             tiles[0][2], tiles[0][3])
            for i, (pbase, n, ub, u8v) in enumerate(tiles):
                o8 = ob8[:, pbase:pbase + n] if ob8 is not None else None
                if i + 1 < len(tiles):
                    pb1, n1, ub1, u81 = tiles[i + 1]
                    def next_a(pb1=pb1, n1=n1, ub1=ub1, u81=u81, i=i):
                        rhs[i + 1] = stage_a(cb, cb8, pb1, n1, ub1, u81)
                else:
                    next_a = None
                stage_b(cb, cb8, pbase, n, ub, u8v, rhs[i][0], rhs[i][1],
                        ob[:, pbase:pbase + n], o8, next_a)

        # ================= phase A: per-chunk levels 9..5 =================
        # software-pipelined: chunk c+1's leaf pass is emitted between chunk
        # c's level-7 and level-6 to hide the chunk-boundary bubble (e9's WAR
        # on chunk c's level-8 reads has cleared by then)
        def leaf_pass(c):
            nleaf = TCH * 512
            base9 = LOFF[9] + c * nleaf
            for hp in range(0, nleaf, 2048):
                ctt = ctpool.tile([FEAT + 1, 2048], bf16, name="ctt", tag="ctt")
                nc.sync.dma_start(ctt[:], ct_d[:, base9 + hp:base9 + hp + 2048])
                for s in range(0, 2048, MMT):
                    leaf_tile(ctt[:, s:s + MMT], e9[:, hp + s:hp + s + MMT],
                              e9m[:, hp + s:hp + s + MMT], MMT)

        leaf_pass(0)
        for c in range(NCHUNK):
            for j, (cb, cb8, ob, ob8) in zip(
                    range(8, 4, -1),
                    [(e9, e9m, e8, e8m), (e8, e8m, e7, e7m),
                     (e7, e7m, e6, e6m), (e6, e6m, None, None)]):
                nj = TCH * (2 ** j)
                if j == 5:
                    ob = em5[:, c * 512:(c + 1) * 512]
                    ob8 = em5m[:, c * 512:(c + 1) * 512]
                else:
                    ob = ob[:, :nj]
                    ob8 = ob8[:, :nj]
                run_level(nj, LOFF[j] + c * nj, cb, cb8, ob, ob8)
                if j == 8 and c + 1 < NCHUNK:
                    leaf_pass(c + 1)

        # ================= phase B: levels 4..0, all trees =================
        chain = [(em5, em5m, e4, e4m), (e4, e4m, e3, e3m), (e3, e3m, e2, e2m),
                 (e2, e2m, e1, e1m), (e1, e1m, e0f, None)]
        for j, (cb, cb8, ob, ob8) in zip(range(4, -1, -1), chain):
            nj = TPC * (2 ** j)
            run_level(nj, LOFF[j], cb, cb8, ob, ob8)

        # ================= output transpose + store =================
        pt = ppB2.tile([H, 2, MMT], f32, name="ptr", tag="pb2")
        nc.tensor.matmul(pt[:, 0, :H], e0f[:], idt[:], is_transpose=True,
                         start=True, stop=True)
        osb = spool.tile([H, MMT], f32, name="osb", tag="osb")
        nc.vector.tensor_copy(osb[:, :H], pt[:, 0, :H])
        nc.sync.dma_start(out_d[:], osb[:, :H])

    nc.compile()
    if not nc.is_finalized():
        nc.finalize()
    return nc


def _prepare(inputs):
    import ml_dtypes

    bf = ml_dtypes.bfloat16
    f8 = ml_dtypes.float8_e4m3

    contents = np.ascontiguousarray(np.asarray(inputs["contents"], np.float32))
    W_u = np.asarray(inputs["W_u"], np.float32)
    b_u = np.asarray(inputs["b_u"], np.float32)
    W_h = np.asarray(inputs["W_h"], np.float32)
    b_h = np.asarray(inputs["b_h"], np.float32)
    W_z = np.asarray(inputs["W_z"], np.float32)
    b_z = np.asarray(inputs["b_z"], np.float32)
    W_r = np.asarray(inputs["W_r"], np.float32)
    b_r = np.asarray(inputs["b_r"], np.float32)
    cw = float(np.asarray(inputs["conv_w"]).reshape(-1)[0])
    cb = float(np.asarray(inputs["conv_b"]).reshape(-1)[0])
    A = cw * cw
    C = cw * cb + cb

    # per-core feature-major contents + ones row, level-major columns
    cts = np.empty((NCORES, FEAT + 1, NPC), np.float32)
    cts[:, FEAT, :] = 1.0
    col = 0
    for j in range(L):
        n = TPC * 2 ** j
        blk = contents[OFF[j]:OFF[j + 1]].reshape(NCORES, n, FEAT)
        cts[:, :FEAT, col:col + n] = blk.transpose(0, 2, 1)
        col += n

    # u-conv weights: pu = (A*cw*W_u)^T c + A*(cw*b_u + cb)
    wu = np.empty((FEAT + 1, H), np.float32)
    wu[:FEAT] = A * cw * W_u
    wu[FEAT] = A * (cw * b_u + cb)

    # r weights (x SW, fp8): blocks rows 0:H=hL, H:2H=hR, 2H:3H=u
    wrlr = np.empty((H, 3, 2, H), np.float32)
    wru = np.empty((H, 3, H), np.float32)
    for m in range(3):
        blk = slice(m * H, (m + 1) * H)
        wrlr[:, m, 0, :] = SW * W_r[0:H, blk]
        wrlr[:, m, 1, :] = SW * W_r[H:2 * H, blk]
        wru[:, m, :] = SW * W_r[2 * H:3 * H, blk]

    # z diff weights: Wd_m = W_z[:, m] - W_z[:, u-gate], rows 0:H=hH, H:2H=hL,
    # 2H:3H=hR, 3H:4H=u
    wdlr = np.empty((H, 3, 2, H), np.float32)
    wdhu = np.empty((H, 3, 2, H), np.float32)
    bd = np.empty((3, H), np.float32)
    for m in range(3):
        Wd = W_z[:, m * H:(m + 1) * H] - W_z[:, 3 * H:4 * H]
        bd[m] = b_z[m * H:(m + 1) * H] - b_z[3 * H:4 * H]
        wdlr[:, m, 0, :] = SW * Wd[H:2 * H]
        wdlr[:, m, 1, :] = SW * Wd[2 * H:3 * H]
        wdhu[:, m, 0, :] = SW * Wd[0:H]
        wdhu[:, m, 1, :] = SW * Wd[3 * H:4 * H]

    # h weights: ph = (0.5*A*cw*W_h)^T rh'  with rh' = (t+1)*hhu
    wh = np.ascontiguousarray((0.5 * A * cw * W_h).reshape(3, H, H).transpose(1, 0, 2))

    bvec = np.zeros((H, 8), np.float32)
    bvec[:, 0] = 0.5 * b_r[0:H]
    bvec[:, 1] = 0.5 * b_r[H:2 * H]
    bvec[:, 2] = 0.5 * b_r[2 * H:3 * H]
    bvec[:, 3] = bd[0]
    bvec[:, 4] = bd[1]
    bvec[:, 5] = bd[2]
    bvec[:, 6] = A * (cw * b_h + cb)

    tanh_pair = bool(np.array_equal(bvec[:, 0], bvec[:, 1]))
    exp_pair = bool(np.array_equal(bvec[:, 4], bvec[:, 5]))

    gdt = f8 if USE_FP8 else bf
    common = {
        "wu": np.ascontiguousarray(wu).astype(bf),
        "wrlr": np.ascontiguousarray(wrlr).astype(gdt),
        "wru": np.ascontiguousarray(wru).astype(gdt),
        "wdlr": np.ascontiguousarray(wdlr).astype(gdt),
        "wdhu": np.ascontiguousarray(wdhu).astype(gdt),
        "wh": wh.astype(bf),
        "bvec": bvec,
        "ident": np.eye(H, dtype=np.float32),
    }
    in_maps = [dict(common, ct=np.ascontiguousarray(cts[c]).astype(bf))
               for c in range(NCORES)]
    return in_maps, tanh_pair, exp_pair


def kernel(**inputs):
    children = np.asarray(inputs["children"])
    cw = float(np.asarray(inputs["conv_w"]).reshape(-1)[0])
    cb = float(np.asarray(inputs["conv_b"]).reshape(-1)[0])
    collapsible = (cw >= 0.0) and (cb >= 0.0)
    if not _children_canonical(children) or not collapsible:
        args = {k: np.asarray(v) for k, v in inputs.items()}
        return _numpy_fallback(**args)

    from concourse.bass_utils import run_bass_kernel_spmd

    in_maps, tanh_pair, exp_pair = _prepare(inputs)
    key = (cw, cb, tanh_pair, exp_pair)
    if key not in _CACHE:
        _CACHE[key] = _build(cw, cb, tanh_pair, exp_pair)
    nc = _CACHE[key]

    res = run_bass_kernel_spmd(nc, in_maps, list(range(NCORES)))
    outs = [res.results[c]["out"] for c in range(NCORES)]
    return np.ascontiguousarray(np.concatenate(outs, axis=0).astype(np.float32))


if __name__ == "__main__":
    print("kernel_v2 module loaded")

